# revision 9
# baseline (speedup 1.0000x reference)
"""CrossAttention Trainium2 kernel (8 NeuronCores).

Reference computation (B=2, N=M=2048, D=1024, H=16, C=64):
    q = rmsnorm(querys @ Wq.T, gq) * C**-0.5       [B,N,D]
    k = rmsnorm(key_feats @ Wk.T, gk)              [B,M,D]
    v = key_feats @ Wv.T                           [B,M,D]
    attn = softmax(mask(q @ k.T per head))         [B,H,N,M]
    out = (attn @ v per head, concat) @ Wo.T + bo  [B,N,D]

Sharding: core = b*4 + j  (b in {0,1}; j in {0..3} owns heads 4j..4j+3,
i.e. a 256-wide slice of D). Per core:
  - project q^T/k^T (d-slice layout [256, 2048]) and v ([2048, 256]) with f32r
    matmuls (contraction over E in the partition dim; host pre-transposes).
  - rmsnorm needs sum-of-squares over the FULL D, so partial sumsq vectors
    ([1,2048] each for q and k) are AllReduced across the 4 cores of each b.
  - attention per head in S^T orientation: S^T[m,n] tiles so softmax's
    reduction (over m) folds into the PV matmul: v is extended with a 65th
    column of ones, making row 64 of the PV accumulator the softmax
    denominator. The mask is folded into the exp as a per-partition bias
    (0 / -1e30). Normalization multiplies by a PE-broadcast reciprocal.
  - out projection produces a partial out^T [1024, 2048] (contraction only
    over this core's d-slice); host sums the 4 partials per b and adds bo.

All matmul operands are float32r (fp32 with mantissa rounded to 11 bits,
full PE rate at N>=256); PSUM accumulation is fp32.
"""

import numpy as np

import concourse.tile as tile
from concourse import bacc, mybir
from concourse.bass_utils import run_bass_kernel_spmd

B, N, M, D, H = 2, 2048, 2048, 1024, 16
C = D // H  # 64, head dim
E = D  # input feature dim
EPS = 1e-6
SCALE = C ** (-0.5)
DS = D // 4  # 256, per-core d-slice
NCORES = 8

f32 = mybir.dt.float32
f32r = mybir.dt.float32r
AF = mybir.ActivationFunctionType

NEG = -1e30


def round_f32r(x: np.ndarray) -> np.ndarray:
    b = np.ascontiguousarray(x, dtype=np.float32).view(np.uint32)
    b = (b + 0x800) & np.uint32(0xFFFFF000)
    return b.view(np.float32)


def build():
    nc = bacc.Bacc(None, target_bir_lowering=False)

    qT_d = nc.declare_dram_parameter("qT", [E, N], f32r, isOutput=False)
    kfT_d = nc.declare_dram_parameter("kfT", [E, M], f32r, isOutput=False)
    wqT_d = nc.declare_dram_parameter("wqT", [E, DS], f32r, isOutput=False)
    wkT_d = nc.declare_dram_parameter("wkT", [E, DS], f32r, isOutput=False)
    wvT_d = nc.declare_dram_parameter("wvT", [E, DS], f32r, isOutput=False)
    woT_d = nc.declare_dram_parameter("woT", [DS, D], f32r, isOutput=False)
    gsq_d = nc.declare_dram_parameter("gsq", [DS], f32r, isOutput=False)
    gsk_d = nc.declare_dram_parameter("gsk", [DS], f32r, isOutput=False)
    mb_d = nc.declare_dram_parameter("mbias", [16, 128], f32, isOutput=False)
    outT_d = nc.declare_dram_parameter("outT", [D, N], f32, isOutput=True)

    with (
        nc.allow_low_precision(reason="f32r matmul operands by design; fp32 PSUM"),
        tile.TileContext(nc) as tc,
    ):
        with (
            tc.tile_pool(name="singles", bufs=1) as singles,
            tc.tile_pool(name="psb", bufs=4) as ppool,
            tc.tile_pool(name="small", bufs=2) as small,
            tc.tile_pool(name="dram", bufs=1, space="DRAM") as dram,
        ):
            # ---- constants / small inputs ----
            ones_f = singles.tile([128, 64], f32)
            nc.vector.memset(ones_f, 1.0)
            ones128 = singles.tile([128, 1], f32r)
            nc.vector.tensor_copy(ones128, ones_f[:, 0:1])
            ones1x64 = singles.tile([1, 64], f32r)
            nc.vector.tensor_copy(ones1x64, ones_f[0:1, :])
            eps_t = singles.tile([1, 1], f32)
            nc.vector.memset(eps_t, EPS)
            invd_t = singles.tile([1, 1], f32)
            nc.vector.memset(invd_t, 1.0 / D)
            gsq_sb = singles.tile([1, DS], f32r)
            nc.sync.dma_start(out=gsq_sb, in_=gsq_d.rearrange("(a n) -> a n", a=1))
            gsk_sb = singles.tile([1, DS], f32r)
            nc.sync.dma_start(out=gsk_sb, in_=gsk_d.rearrange("(a n) -> a n", a=1))
            mb_sb = singles.tile([128, 16], f32)
            nc.sync.dma_start(out=mb_sb, in_=mb_d.rearrange("t p -> p t"))

            # ---- weights ----
            wq_sb = singles.tile([128, 8, DS], f32r)
            wk_sb = singles.tile([128, 8, DS], f32r)
            wv_sb = singles.tile([128, 8, DS], f32r)
            for et in range(8):
                nc.sync.dma_start(out=wq_sb[:, et, :], in_=wqT_d[et * 128 : et * 128 + 128, :])
                nc.sync.dma_start(out=wk_sb[:, et, :], in_=wkT_d[et * 128 : et * 128 + 128, :])
                nc.sync.dma_start(out=wv_sb[:, et, :], in_=wvT_d[et * 128 : et * 128 + 128, :])

            # ---- persistent activations (q/k normalized IN PLACE later) ----
            qT = singles.tile([128, 2, 4, 512], f32r)  # [p, dt, nb, n]
            kT = singles.tile([128, 2, 4, 512], f32r)  # [p, dt, mb, m]
            v_sb = singles.tile([128, 16, 4, C + 1], f32r)  # [m_p, mt, h, c|ones]
            xT = singles.tile([128, 2, 4, 512], f32r)  # [p, dt, nb, n]
            nc.vector.tensor_copy(
                v_sb[:, :, :, C], ones_f.rearrange("p (a b) -> p a b", a=16)
            )

            cc_in = dram.tile([2 * 2048], f32)
            cc_out = dram.tile([2 * 2048], f32)

            with (
                tc.tile_pool(name="inblk", bufs=2) as inblk,
                tc.tile_pool(name="sq", bufs=2) as sqpool,
                tc.tile_pool(name="normsb", bufs=1) as normsb,
                tc.tile_pool(name="projps", bufs=2, space="PSUM") as projps,
                tc.tile_pool(name="vps", bufs=2, space="PSUM") as vps,
                tc.tile_pool(name="ssps", bufs=2, space="PSUM") as ssps,
            ):
                # ---- phase 1+2: q/k/v projections + partial sumsq ----
                for which in ("q", "k"):
                    src_d = qT_d if which == "q" else kfT_d
                    w_sb = wq_sb if which == "q" else wk_sb
                    dst = qT if which == "q" else kT
                    ss_off = 0 if which == "q" else 2048
                    for nb in range(4):
                        blk = inblk.tile([128, 8, 512], f32r, tag="blk")
                        for et in range(8):
                            nc.sync.dma_start(
                                out=blk[:, et, :],
                                in_=src_d[et * 128 : et * 128 + 128, nb * 512 : nb * 512 + 512],
                            )
                        ss_ps = ssps.tile([1, 512], f32, tag="ss")
                        for dt in range(2):
                            ps = projps.tile([128, 512], f32, tag="proj")
                            for et in range(8):
                                nc.tensor.matmul(
                                    ps,
                                    w_sb[:, et, dt * 128 : dt * 128 + 128],
                                    blk[:, et, :],
                                    start=(et == 0),
                                    stop=(et == 7),
                                )
                            nc.vector.tensor_copy(dst[:, dt, nb, :], ps)
                            sq = sqpool.tile([128, 512], f32r, tag="sq")
                            nc.vector.tensor_mul(sq, dst[:, dt, nb, :], dst[:, dt, nb, :])
                            nc.tensor.matmul(
                                ss_ps, ones128, sq, start=(dt == 0), stop=(dt == 1)
                            )
                        ss_sb = small.tile([1, 512], f32, tag="ss_sb")
                        nc.scalar.copy(ss_sb, ss_ps)
                        nc.sync.dma_start(
                            out=cc_in[ss_off + nb * 512 : ss_off + nb * 512 + 512].rearrange(
                                "(a n) -> a n", a=1
                            ),
                            in_=ss_sb,
                        )
                        if which == "k":
                            # v projection for this m-block
                            for mt in range(4):
                                psv = vps.tile([128, 256], f32, tag="v")
                                for et in range(8):
                                    nc.tensor.matmul(
                                        psv,
                                        blk[:, et, mt * 128 : mt * 128 + 128],
                                        wv_sb[:, et, :],
                                        start=(et == 0),
                                        stop=(et == 7),
                                    )
                                nc.vector.tensor_copy(
                                    v_sb[:, nb * 4 + mt, :, 0:C],
                                    psv.rearrange("p (h c) -> p h c", c=C),
                                )

                # ---- collective: AllReduce partial sumsq within each b-group ----
                nc.gpsimd.collective_compute(
                    "AllReduce",
                    mybir.AluOpType.add,
                    replica_groups=[[0, 1, 2, 3], [4, 5, 6, 7]],
                    ins=[cc_in.opt()],
                    outs=[cc_out.opt()],
                )

                # ---- norm finalize (in place): x^T *= gs[d] * rstd[n] ----
                for which, fin, gs_sb, off in (
                    ("q", qT, gsq_sb, 0),
                    ("k", kT, gsk_sb, 2048),
                ):
                    ss_full = normsb.tile([1, 2048], f32, tag="ssf")
                    nc.sync.dma_start(
                        out=ss_full,
                        in_=cc_out[off : off + 2048].rearrange("(a n) -> a n", a=1),
                    )
                    std_t = normsb.tile([1, 2048], f32, tag="std")
                    nc.scalar.activation(
                        std_t, ss_full, AF.Sqrt, bias=eps_t, scale=invd_t
                    )
                    rstd = normsb.tile([1, 2048], f32r, tag="rstd")
                    nc.vector.reciprocal(rstd, std_t)
                    for dt in range(2):
                        for nb in range(4):
                            bc = projps.tile([128, 512], f32, tag="proj")
                            nc.tensor.matmul(
                                bc,
                                gs_sb[:, dt * 128 : dt * 128 + 128],
                                rstd[:, nb * 512 : nb * 512 + 512],
                                start=True,
                                stop=True,
                            )
                            nc.vector.tensor_mul(fin[:, dt, nb, :], fin[:, dt, nb, :], bc)

            # ---- phase 3: attention (4 heads) ----
            with (
                tc.tile_pool(name="sps", bufs=2, space="PSUM") as sps,
                tc.tile_pool(name="ops", bufs=2, space="PSUM") as ops,
                tc.tile_pool(name="bcps", bufs=1, space="PSUM") as bcps,
                tc.tile_pool(name="outps", bufs=3, space="PSUM") as outps,
            ):
                for h in range(4):
                    dt, off = h // 2, (h % 2) * C
                    for nb in range(4):
                        ps_o = ops.tile([C + 1, 512], f32, tag="o")
                        for mt in range(16):
                            ps_s = sps.tile([128, 512], f32, tag="s")
                            nc.tensor.matmul(
                                ps_s,
                                kT[off : off + C, dt, mt // 4, (mt % 4) * 128 : (mt % 4) * 128 + 128],
                                qT[off : off + C, dt, nb, :],
                                start=True,
                                stop=True,
                            )
                            p_sb = ppool.tile([128, 512], f32r, tag="p")
                            nc.scalar.activation(
                                p_sb, ps_s, AF.Exp, bias=mb_sb[:, mt : mt + 1], scale=1.0
                            )
                            nc.tensor.matmul(
                                ps_o, v_sb[:, mt, h, :], p_sb,
                                start=(mt == 0), stop=(mt == 15),
                            )
                        rd = small.tile([1, 512], f32r, tag="rd")
                        nc.vector.reciprocal(rd, ps_o[C : C + 1, :])
                        bc = bcps.tile([C, 512], f32, tag="bc")
                        nc.tensor.matmul(bc, ones1x64, rd, start=True, stop=True)
                        bc_sb = ppool.tile([C, 512], f32, tag="bcsb")
                        nc.scalar.copy(bc_sb, bc)
                        nc.vector.tensor_mul(
                            xT[off : off + C, dt, nb, :], ps_o[0:C, :], bc_sb
                        )

                # ---- phase 4: out projection (partial over d-slice) ----
                wo_sb = singles.tile([128, 2, D], f32r)
                for dc in range(2):
                    nc.sync.dma_start(
                        out=wo_sb[:, dc, :], in_=woT_d[dc * 128 : dc * 128 + 128, :]
                    )
                for nb in range(4):
                    for ot in range(8):
                        ps = outps.tile([128, 512], f32, tag="out")
                        for dc in range(2):
                            nc.tensor.matmul(
                                ps,
                                wo_sb[:, dc, ot * 128 : ot * 128 + 128],
                                xT[:, dc, nb, :],
                                start=(dc == 0),
                                stop=(dc == 1),
                            )
                        out_sb = ppool.tile([128, 512], f32, tag="osb")
                        nc.scalar.copy(out_sb, ps)
                        nc.sync.dma_start(
                            out=outT_d[ot * 128 : ot * 128 + 128, nb * 512 : nb * 512 + 512],
                            in_=out_sb,
                        )

    nc.finalize()
    return nc


_NC_CACHE = None


def _get_nc():
    global _NC_CACHE
    if _NC_CACHE is None:
        _NC_CACHE = build()
    return _NC_CACHE


def make_in_maps(querys, key_feats, mask, Wq, Wk, Wv, gq, gk, Wo, bo):
    querys = np.asarray(querys, dtype=np.float32)
    key_feats = np.asarray(key_feats, dtype=np.float32)
    mask = np.asarray(mask)
    gq = np.asarray(gq, dtype=np.float32)
    gk = np.asarray(gk, dtype=np.float32)

    qT = [round_f32r(querys[b].T) for b in range(B)]
    kfT = [round_f32r(key_feats[b].T) for b in range(B)]
    mb = [
        np.where(mask[b] == 0, np.float32(NEG), np.float32(0.0))
        .astype(np.float32)
        .reshape(16, 128)
        for b in range(B)
    ]
    wqT, wkT, wvT, woT, gsq, gsk = [], [], [], [], [], []
    for j in range(4):
        dsl = slice(j * DS, (j + 1) * DS)
        wqT.append(round_f32r(np.asarray(Wq)[dsl].T))
        wkT.append(round_f32r(np.asarray(Wk)[dsl].T))
        wvT.append(round_f32r(np.asarray(Wv)[dsl].T))
        woT.append(round_f32r(np.asarray(Wo)[:, dsl].T))
        gsq.append(round_f32r(gq[dsl] * np.float32(SCALE)))
        gsk.append(round_f32r(gk[dsl]))

    in_maps = []
    for cid in range(NCORES):
        b, j = cid // 4, cid % 4
        in_maps.append(
            {
                "qT": qT[b],
                "kfT": kfT[b],
                "wqT": wqT[j],
                "wkT": wkT[j],
                "wvT": wvT[j],
                "woT": woT[j],
                "gsq": gsq[j],
                "gsk": gsk[j],
                "mbias": mb[b],
            }
        )
    return in_maps


def assemble(results, bo):
    bo = np.asarray(bo, dtype=np.float32)
    out = np.zeros((B, N, D), dtype=np.float32)
    for cid in range(NCORES):
        b = cid // 4
        out[b] += results[cid]["outT"].T
    out += bo
    return out


def kernel(querys, key_feats, mask, Wq, Wk, Wv, gq, gk, Wo, bo):
    nc = _get_nc()
    in_maps = make_in_maps(querys, key_feats, mask, Wq, Wk, Wv, gq, gk, Wo, bo)
    res = run_bass_kernel_spmd(nc, in_maps, list(range(NCORES)))
    return assemble(res.results, bo)


# revision 23
# speedup vs baseline: 1.0893x; 1.0893x over previous
"""CrossAttention Trainium2 kernel (8 NeuronCores).

Reference computation (B=2, N=M=2048, D=1024, H=16, C=64):
    q = rmsnorm(querys @ Wq.T, gq) * C**-0.5       [B,N,D]
    k = rmsnorm(key_feats @ Wk.T, gk)              [B,M,D]
    v = key_feats @ Wv.T                           [B,M,D]
    attn = softmax(mask(q @ k.T per head))         [B,H,N,M]
    out = (attn @ v per head, concat) @ Wo.T + bo  [B,N,D]

Sharding: core = b*4 + j (b in {0,1}; j in {0..3} owns heads 4j..4j+3 = a
256-wide slice of D). Host pre-transposes inputs/weights, folds gq*scale /
gk into Wq / Wk rows, and pre-rounds everything to f32r (fp32 with 11-bit
mantissa -> full PE rate). Per core:

  - q'^T / k'^T projections in d-slice layout [256, 2048] (contraction over
    E in the partition dim), v in [2048, 256]. q' = gs_q * q_raw etc.
  - rmsnorm sum-of-squares over the FULL D: per-core partial sumsq is
    computed by a matmul against a 1/gs^2-weighted column (compensating the
    folded gains) and AllReduced (8KB) across the 4 cores of each b; the
    collectives are emitted right after their producing phase so they hide
    behind the next projection.
  - rstd chains run lane-parallel in [128,16] layout. rstd_k is NOT applied
    to k': in the S^T = k'q'^T orientation the softmax logit scale rstd_k[m]
    is per-partition, so it folds into the exp ACTIVATE as its scale operand
    (and the mask as its bias: 0 / -1e30). rstd_q is applied to q' via a
    PE-transpose into row layout + ones outer-product broadcast.
  - attention per head: for each m-tile, 4 QK matmuls (one per 512-wide
    n-block, shared k stationary) -> batched exp -> 4 PV matmuls into a
    4-bank accumulator. v carries a 65th column of ones so row 64 of the
    accumulator is the softmax denominator (reciprocal_approx_fast + ones
    outer-product broadcast + one multiply normalizes the head output).
  - out projection produces a partial out^T [1024, 2048] (contraction over
    this core's d-slice only); the host sums 4 partials per b and adds bo.
"""

import os

import numpy as np

import concourse.tile as tile
from concourse import bacc, mybir
from concourse.bass_utils import run_bass_kernel_spmd

DEBUG = bool(os.environ.get("BASSK_DEBUG"))

B, N, M, D, H = 2, 2048, 2048, 1024, 16
C = D // H  # 64, head dim
E = D  # input feature dim
EPS = 1e-6
SCALE = C ** (-0.5)
DS = D // 4  # 256, per-core d-slice
NCORES = 8

f32 = mybir.dt.float32
f32r = mybir.dt.float32r
AF = mybir.ActivationFunctionType

NEG = -1e30


def round_f32r(x: np.ndarray) -> np.ndarray:
    b = np.ascontiguousarray(x, dtype=np.float32).view(np.uint32)
    b = (b + 0x800) & np.uint32(0xFFFFF000)
    return b.view(np.float32)


def build():
    nc = bacc.Bacc(None, target_bir_lowering=False)

    qT_d = nc.declare_dram_parameter("qT", [E, N], f32r, isOutput=False)
    kfT_d = nc.declare_dram_parameter("kfT", [E, M], f32r, isOutput=False)
    wqT_d = nc.declare_dram_parameter("wqT", [E, DS], f32r, isOutput=False)
    wkT_d = nc.declare_dram_parameter("wkT", [E, DS], f32r, isOutput=False)
    wvT_d = nc.declare_dram_parameter("wvT", [E, DS], f32r, isOutput=False)
    woT_d = nc.declare_dram_parameter("woT", [DS, D], f32r, isOutput=False)
    ig2q_d = nc.declare_dram_parameter("ig2q", [2, 128], f32r, isOutput=False)
    ig2k_d = nc.declare_dram_parameter("ig2k", [2, 128], f32r, isOutput=False)
    mb_d = nc.declare_dram_parameter("mbias", [16, 128], f32, isOutput=False)
    outT_d = nc.declare_dram_parameter("outT", [D, N], f32, isOutput=True)
    if DEBUG:
        dbg_q = nc.declare_dram_parameter("dbg_q", [128, 2, 4, 512], f32r, isOutput=True)
        dbg_k = nc.declare_dram_parameter("dbg_k", [128, 2, 4, 512], f32r, isOutput=True)
        dbg_v = nc.declare_dram_parameter("dbg_v", [128, 16, 4, C + 1], f32r, isOutput=True)
        dbg_x = nc.declare_dram_parameter("dbg_x", [128, 2, 4, 512], f32r, isOutput=True)
        dbg_rk = nc.declare_dram_parameter("dbg_rk", [128, 16], f32, isOutput=True)
        dbg_rq = nc.declare_dram_parameter("dbg_rq", [1, 2048], f32, isOutput=True)
        dbg_s = nc.declare_dram_parameter("dbg_s", [128, 2, 512], f32, isOutput=True)
        dbg_p = nc.declare_dram_parameter("dbg_p", [128, 2, 512], f32r, isOutput=True)
        dbg_o = nc.declare_dram_parameter("dbg_o", [C + 1, 4, 512], f32, isOutput=True)
        dbg_rd = nc.declare_dram_parameter("dbg_rd", [1, 512], f32, isOutput=True)
        dbg_bc = nc.declare_dram_parameter("dbg_bc", [C, 512], f32, isOutput=True)

    with (
        nc.allow_low_precision(reason="f32r matmul operands by design; fp32 PSUM"),
        tile.TileContext(nc) as tc,
    ):
        with (
            tc.tile_pool(name="singles", bufs=1) as singles,
            tc.tile_pool(name="blk", bufs=1 if DEBUG else 2) as blkpool,
            tc.tile_pool(name="sq", bufs=2) as sqpool,
            tc.tile_pool(name="psb", bufs=3) as ppool,
            tc.tile_pool(name="small", bufs=2) as small,
            tc.tile_pool(name="dram", bufs=1, space="DRAM") as dram,
        ):
            # ---- constants / small inputs ----
            ones_f = singles.tile([128, 64], f32)
            nc.vector.memset(ones_f, 1.0)
            ones1x64 = singles.tile([1, 64], f32)
            nc.vector.memset(ones1x64, 1.0)
            ones1x128 = singles.tile([1, 128], f32)
            nc.vector.memset(ones1x128, 1.0)
            eps_t = singles.tile([128, 1], f32)
            nc.vector.memset(eps_t, EPS)
            invd_t = singles.tile([128, 1], f32)
            nc.vector.memset(invd_t, 1.0 / D)
            ig2q_sb = singles.tile([128, 2], f32r)
            nc.sync.dma_start(out=ig2q_sb, in_=ig2q_d.rearrange("t p -> p t"))
            ig2k_sb = singles.tile([128, 2], f32r)
            nc.sync.dma_start(out=ig2k_sb, in_=ig2k_d.rearrange("t p -> p t"))
            mb_sb = singles.tile([128, 16], f32)
            nc.sync.dma_start(out=mb_sb, in_=mb_d.rearrange("t p -> p t"))

            wq_sb = singles.tile([128, 8, DS], f32r)
            wk_sb = singles.tile([128, 8, DS], f32r)
            wv_sb = singles.tile([128, 8, DS], f32r)

            # ---- persistent activations ----
            qT = singles.tile([128, 2, 4, 512], f32r)  # [p, dt, nb, n]
            kT = singles.tile([128, 2, 4, 512], f32r)  # [p, dt, mb, m]
            v_sb = singles.tile([128, 16, 4, C + 1], f32r)  # [m_p, mt, h, c|ones]
            xT = singles.tile([128, 2, 4, 512], f32r)  # [p, dt, nb, n]
            nc.vector.tensor_copy(
                v_sb[:, :, :, C], ones_f.rearrange("p (a b) -> p a b", a=16)
            )

            ccq_in = dram.tile([2048], f32)
            ccq_out = dram.tile([2048], f32)
            cck_in = dram.tile([2048], f32)
            cck_out = dram.tile([2048], f32)

            def projection(src_d, w_sb, dst, ig2_sb, cc_in_t):
                """dst[dt, nb] = W'^T-slice @ src-block; partial sumsq -> cc_in."""
                for nb in range(4):
                    blk = blkpool.tile([128, 8, 512], f32r, tag="blk")
                    for et in range(8):
                        nc.sync.dma_start(
                            out=blk[:, et, :],
                            in_=src_d[et * 128 : et * 128 + 128, nb * 512 : nb * 512 + 512],
                        )
                    ss_ps = ssps.tile([1, 512], f32, tag="ss")
                    for dt in range(2):
                        ps = projps.tile([128, 512], f32, tag="proj")
                        for et in range(8):
                            nc.tensor.matmul(
                                ps,
                                w_sb[:, et, dt * 128 : dt * 128 + 128],
                                blk[:, et, :],
                                start=(et == 0),
                                stop=(et == 7),
                            )
                        nc.vector.tensor_copy(dst[:, dt, nb, :], ps)
                        sq = sqpool.tile([128, 512], f32r, tag="sq")
                        nc.vector.tensor_mul(sq, dst[:, dt, nb, :], dst[:, dt, nb, :])
                        nc.tensor.matmul(
                            ss_ps,
                            ig2_sb[:, dt : dt + 1],
                            sq,
                            start=(dt == 0),
                            stop=(dt == 1),
                            skip_group_check=True,
                        )
                    ss_sb = small.tile([1, 512], f32, tag="ss_sb")
                    nc.scalar.copy(ss_sb, ss_ps)
                    nc.sync.dma_start(
                        out=cc_in_t[nb * 512 : nb * 512 + 512].rearrange(
                            "(a n) -> a n", a=1
                        ),
                        in_=ss_sb,
                    )

            def rstd128(cc_out_t, tag):
                """[128,16] lane-parallel rstd chain: p,t -> 1/sqrt(ss/D+eps)."""
                ss128 = small.tile([128, 16], f32, tag=f"ss128{tag}")
                nc.sync.dma_start(
                    out=ss128, in_=cc_out_t.rearrange("(t p) -> p t", p=128)
                )
                std = small.tile([128, 16], f32, tag=f"std{tag}")
                nc.scalar.activation(std, ss128, AF.Sqrt, bias=eps_t, scale=invd_t)
                r = singles.tile([128, 16], f32)
                nc.vector.reciprocal_approx_fast(out=r, in_=std)
                return r

            with (
                tc.tile_pool(name="projps", bufs=2, space="PSUM") as projps,
                tc.tile_pool(name="vps", bufs=2, space="PSUM") as vps,
                tc.tile_pool(name="ssps", bufs=2, space="PSUM") as ssps,
            ):
                # ---- q projection, then its collective (hidden behind k/v) ----
                for et in range(8):
                    nc.sync.dma_start(out=wq_sb[:, et, :], in_=wqT_d[et * 128 : et * 128 + 128, :])
                projection(qT_d, wq_sb, qT, ig2q_sb, ccq_in)
                nc.gpsimd.collective_compute(
                    "AllReduce",
                    mybir.AluOpType.add,
                    replica_groups=[[0, 1, 2, 3], [4, 5, 6, 7]],
                    ins=[ccq_in.opt()],
                    outs=[ccq_out.opt()],
                )

                # ---- k projection, then its collective (hidden behind v) ----
                for et in range(8):
                    nc.sync.dma_start(out=wk_sb[:, et, :], in_=wkT_d[et * 128 : et * 128 + 128, :])
                projection(kfT_d, wk_sb, kT, ig2k_sb, cck_in)
                nc.gpsimd.collective_compute(
                    "AllReduce",
                    mybir.AluOpType.add,
                    replica_groups=[[0, 1, 2, 3], [4, 5, 6, 7]],
                    ins=[cck_in.opt()],
                    outs=[cck_out.opt()],
                )

                # ---- v projection (kfT re-streamed) ----
                for et in range(8):
                    nc.sync.dma_start(out=wv_sb[:, et, :], in_=wvT_d[et * 128 : et * 128 + 128, :])
                for mb in range(4):
                    blk = blkpool.tile([128, 8, 512], f32r, tag="blk")
                    for et in range(8):
                        nc.sync.dma_start(
                            out=blk[:, et, :],
                            in_=kfT_d[et * 128 : et * 128 + 128, mb * 512 : mb * 512 + 512],
                        )
                    for mt in range(4):
                        psv = vps.tile([128, 256], f32, tag="v")
                        for et in range(8):
                            nc.tensor.matmul(
                                psv,
                                blk[:, et, mt * 128 : mt * 128 + 128],
                                wv_sb[:, et, :],
                                start=(et == 0),
                                stop=(et == 7),
                            )
                        nc.vector.tensor_copy(
                            v_sb[:, mb * 4 + mt, :, 0:C],
                            psv.rearrange("p (h c) -> p h c", c=C),
                        )

                # ---- rstd_k: [128,16] lane-parallel; feeds exp scale directly ----
                rstdk = rstd128(cck_out, "k")

                # ---- rstd_q: row layout [1, 2048] for the bcast outer-products ----
                ssq_row = singles.tile([1, 2048], f32)
                nc.sync.dma_start(
                    out=ssq_row, in_=ccq_out.rearrange("(a n) -> a n", a=1)
                )
                stdq_row = singles.tile([1, 2048], f32)
                nc.scalar.activation(
                    stdq_row, ssq_row, AF.Sqrt, bias=eps_t[0:1, :], scale=invd_t[0:1, :]
                )
                rs_row = singles.tile([1, 2048], f32)
                nc.vector.reciprocal_approx_fast(out=rs_row, in_=stdq_row)
                # q finalize: qT[d, n] *= rstd_q[n] via ones outer-product bcast
                for nb in range(4):
                    bcq = projps.tile([128, 512], f32, tag="proj")
                    nc.tensor.matmul(
                        bcq,
                        ones1x128,
                        rs_row[:, nb * 512 : nb * 512 + 512],
                        start=True,
                        stop=True,
                    )
                    for dt in range(2):
                        nc.vector.tensor_mul(qT[:, dt, nb, :], qT[:, dt, nb, :], bcq)

            # ---- phase 3: attention (4 heads) ----
            with (
                tc.tile_pool(name="sps", bufs=2, space="PSUM") as spool,
                tc.tile_pool(name="ops", bufs=1, space="PSUM") as opool,
            ):
                for h in range(4):
                    dt, off = h // 2, (h % 2) * C
                    o4 = opool.tile([C + 1, 4, 512], f32, tag="o4")
                    for mt in range(16):
                        kT_lhs = kT[
                            off : off + C, dt, mt // 4, (mt % 4) * 128 : (mt % 4) * 128 + 128
                        ]
                        pv_halves = []
                        for half in range(2):
                            s2 = spool.tile([128, 2, 512], f32, tag="s2")
                            for i in range(2):
                                nbi = half * 2 + i
                                nc.tensor.matmul(
                                    s2[:, i, :],
                                    kT_lhs,
                                    qT[off : off + C, dt, nbi, :],
                                    start=True,
                                    stop=True,
                                )
                            p2 = ppool.tile([128, 2, 512], f32r, tag="p")
                            nc.scalar.activation(
                                p2, s2, AF.Exp,
                                bias=mb_sb[:, mt : mt + 1],
                                scale=rstdk[:, mt : mt + 1],
                            )
                            pv_halves.append(p2)
                            if DEBUG and h == 0 and mt == 0 and half == 0:
                                s_sb = ppool.tile([128, 2, 512], f32, tag="dbgs")
                                nc.vector.tensor_copy(s_sb, s2)
                                nc.sync.dma_start(out=dbg_s[:], in_=s_sb)
                                nc.sync.dma_start(out=dbg_p[:], in_=p2)
                        for nbi in range(4):
                            nc.tensor.matmul(
                                o4[:, nbi, :],
                                v_sb[:, mt, h, :],
                                pv_halves[nbi // 2][:, nbi % 2, :],
                                start=(mt == 0),
                                stop=(mt == 15),
                                skip_group_check=True,
                            )
                    if DEBUG and h == 0:
                        o_sb = singles.tile([C + 1, 4, 512], f32)
                        nc.vector.tensor_copy(o_sb, o4)
                        nc.sync.dma_start(out=dbg_o[:], in_=o_sb)
                    for nb in range(4):
                        den_sb = small.tile([1, 512], f32, tag="den")
                        nc.vector.tensor_copy(den_sb, o4[C : C + 1, nb, :])
                        rd = small.tile([1, 512], f32, tag="rd")
                        nc.vector.reciprocal_approx_fast(out=rd, in_=den_sb)
                        bc = spool.tile([128, 2, 512], f32, tag="s2")
                        nc.tensor.matmul(bc[0:C, 0, :], ones1x64, rd, start=True, stop=True)
                        bc_sb = ppool.tile([C, 512], f32, tag="bcsb")
                        nc.vector.tensor_copy(bc_sb, bc[0:C, 0, :])
                        if DEBUG and h == 0 and nb == 0:
                            nc.sync.dma_start(out=dbg_rd[:], in_=rd)
                            nc.sync.dma_start(out=dbg_bc[:], in_=bc_sb)
                        nc.vector.tensor_mul(
                            xT[off : off + C, dt, nb, :], o4[0:C, nb, :], bc_sb
                        )

            if DEBUG:
                nc.sync.dma_start(out=dbg_q[:], in_=qT)
                nc.sync.dma_start(out=dbg_k[:], in_=kT)
                nc.sync.dma_start(out=dbg_v[:], in_=v_sb)
                nc.sync.dma_start(out=dbg_x[:], in_=xT)
                nc.sync.dma_start(out=dbg_rk[:], in_=rstdk)
                nc.sync.dma_start(out=dbg_rq[:], in_=rs_row)

            # ---- phase 4: out projection (partial over d-slice) ----
            with tc.tile_pool(name="outps", bufs=3, space="PSUM") as outps:
                wo_sb = singles.tile([128, 2, D], f32r)
                for dc in range(2):
                    nc.sync.dma_start(
                        out=wo_sb[:, dc, :], in_=woT_d[dc * 128 : dc * 128 + 128, :]
                    )
                for nb in range(4):
                    for ot in range(8):
                        ps = outps.tile([128, 512], f32, tag="out")
                        for dc in range(2):
                            nc.tensor.matmul(
                                ps,
                                wo_sb[:, dc, ot * 128 : ot * 128 + 128],
                                xT[:, dc, nb, :],
                                start=(dc == 0),
                                stop=(dc == 1),
                            )
                        out_sb = ppool.tile([128, 512], f32, tag="osb")
                        nc.scalar.copy(out_sb, ps)
                        nc.sync.dma_start(
                            out=outT_d[ot * 128 : ot * 128 + 128, nb * 512 : nb * 512 + 512],
                            in_=out_sb,
                        )

    nc.finalize()
    return nc


_NC_CACHE = None


def _get_nc():
    global _NC_CACHE
    if _NC_CACHE is None:
        _NC_CACHE = build()
    return _NC_CACHE


def make_in_maps(querys, key_feats, mask, Wq, Wk, Wv, gq, gk, Wo, bo):
    querys = np.asarray(querys, dtype=np.float32)
    key_feats = np.asarray(key_feats, dtype=np.float32)
    mask = np.asarray(mask)
    gq = np.asarray(gq, dtype=np.float32)
    gk = np.asarray(gk, dtype=np.float32)

    gsq_full = gq * np.float32(SCALE)  # folded into Wq rows
    gsk_full = gk.astype(np.float32)  # folded into Wk rows
    Wq_f = np.asarray(Wq, dtype=np.float32) * gsq_full[:, None]
    Wk_f = np.asarray(Wk, dtype=np.float32) * gsk_full[:, None]

    qT = [round_f32r(querys[b].T) for b in range(B)]
    kfT = [round_f32r(key_feats[b].T) for b in range(B)]
    mb = [
        np.where(mask[b] == 0, np.float32(NEG), np.float32(0.0))
        .astype(np.float32)
        .reshape(16, 128)
        for b in range(B)
    ]
    wqT, wkT, wvT, woT, ig2q, ig2k = [], [], [], [], [], []
    for j in range(4):
        dsl = slice(j * DS, (j + 1) * DS)
        wqT.append(round_f32r(Wq_f[dsl].T))
        wkT.append(round_f32r(Wk_f[dsl].T))
        wvT.append(round_f32r(np.asarray(Wv)[dsl].T))
        woT.append(round_f32r(np.asarray(Wo)[:, dsl].T))
        # sumsq compensation: raw sumsq = sum_d (q'_d)^2 / gs_d^2
        ig2q.append(round_f32r((1.0 / gsq_full[dsl] ** 2).reshape(2, 128)))
        ig2k.append(round_f32r((1.0 / gsk_full[dsl] ** 2).reshape(2, 128)))

    in_maps = []
    for cid in range(NCORES):
        b, j = cid // 4, cid % 4
        in_maps.append(
            {
                "qT": qT[b],
                "kfT": kfT[b],
                "wqT": wqT[j],
                "wkT": wkT[j],
                "wvT": wvT[j],
                "woT": woT[j],
                "ig2q": ig2q[j],
                "ig2k": ig2k[j],
                "mbias": mb[b],
            }
        )
    return in_maps


def assemble(results, bo):
    bo = np.asarray(bo, dtype=np.float32)
    out = np.zeros((B, N, D), dtype=np.float32)
    for cid in range(NCORES):
        b = cid // 4
        out[b] += results[cid]["outT"].T
    out += bo
    return out


def kernel(querys, key_feats, mask, Wq, Wk, Wv, gq, gk, Wo, bo):
    nc = _get_nc()
    in_maps = make_in_maps(querys, key_feats, mask, Wq, Wk, Wv, gq, gk, Wo, bo)
    res = run_bass_kernel_spmd(nc, in_maps, list(range(NCORES)))
    return assemble(res.results, bo)


# revision 32
# speedup vs baseline: 1.1518x; 1.0574x over previous
"""CrossAttention Trainium2 kernel (8 NeuronCores).

Reference computation (B=2, N=M=2048, D=1024, H=16, C=64):
    q = rmsnorm(querys @ Wq.T, gq) * C**-0.5       [B,N,D]
    k = rmsnorm(key_feats @ Wk.T, gk)              [B,M,D]
    v = key_feats @ Wv.T                           [B,M,D]
    attn = softmax(mask(q @ k.T per head))         [B,H,N,M]
    out = (attn @ v per head, concat) @ Wo.T + bo  [B,N,D]

Sharding: core = b*4 + j (b in {0,1}; j in {0..3} owns heads 4j..4j+3 = a
256-wide slice of D). Host pre-transposes inputs/weights, folds gq*scale /
gk into Wq / Wk rows, and pre-rounds everything to f32r (fp32 with 11-bit
mantissa -> full PE rate). Per core:

  - q'^T / k'^T projections in d-slice layout [256, 2048] (contraction over
    E in the partition dim), v in [2048, 256]. q' = gs_q * q_raw etc.
  - rmsnorm sum-of-squares over the FULL D: per-core partial sumsq is
    computed by a matmul against a 1/gs^2-weighted column (compensating the
    folded gains) and AllReduced (8KB) across the 4 cores of each b; the
    collectives are emitted right after their producing phase so they hide
    behind the next projection.
  - rstd chains run lane-parallel in [128,16] layout. rstd_k is NOT applied
    to k': in the S^T = k'q'^T orientation the softmax logit scale rstd_k[m]
    is per-partition, so it folds into the exp ACTIVATE as its scale operand
    (and the mask as its bias: 0 / -1e30). rstd_q is applied to q' via a
    PE-transpose into row layout + ones outer-product broadcast.
  - attention per head: for each m-tile, 4 QK matmuls (one per 512-wide
    n-block, shared k stationary) -> batched exp -> 4 PV matmuls into a
    4-bank accumulator. v carries a 65th column of ones so row 64 of the
    accumulator is the softmax denominator (reciprocal_approx_fast + ones
    outer-product broadcast + one multiply normalizes the head output).
  - out projection produces a partial out^T [1024, 2048] (contraction over
    this core's d-slice only); the host sums 4 partials per b and adds bo.
"""

import os

import numpy as np

import concourse.tile as tile
from concourse import bacc, mybir
from concourse.bass_utils import run_bass_kernel_spmd

DEBUG = bool(os.environ.get("BASSK_DEBUG"))

B, N, M, D, H = 2, 2048, 2048, 1024, 16
C = D // H  # 64, head dim
E = D  # input feature dim
EPS = 1e-6
SCALE = C ** (-0.5)
DS = D // 4  # 256, per-core d-slice
NCORES = 8

f32 = mybir.dt.float32
f32r = mybir.dt.float32r
AF = mybir.ActivationFunctionType

NEG = -1e30


def round_f32r(x: np.ndarray) -> np.ndarray:
    b = np.ascontiguousarray(x, dtype=np.float32).view(np.uint32)
    b = (b + 0x800) & np.uint32(0xFFFFF000)
    return b.view(np.float32)


def build():
    nc = bacc.Bacc(None, target_bir_lowering=False)

    qT_d = nc.declare_dram_parameter("qT", [E, N], f32r, isOutput=False)
    kfT_d = nc.declare_dram_parameter("kfT", [E, M], f32r, isOutput=False)
    wqT_d = nc.declare_dram_parameter("wqT", [E, DS], f32r, isOutput=False)
    wkT_d = nc.declare_dram_parameter("wkT", [E, DS], f32r, isOutput=False)
    wvT_d = nc.declare_dram_parameter("wvT", [E, DS], f32r, isOutput=False)
    woT_d = nc.declare_dram_parameter("woT", [DS, D], f32r, isOutput=False)
    ig2q_d = nc.declare_dram_parameter("ig2q", [2, 128], f32r, isOutput=False)
    ig2k_d = nc.declare_dram_parameter("ig2k", [2, 128], f32r, isOutput=False)
    mb_d = nc.declare_dram_parameter("mbias", [16, 128], f32, isOutput=False)
    outT_d = nc.declare_dram_parameter("outT", [D, N], f32, isOutput=True)
    if DEBUG:
        dbg_q = nc.declare_dram_parameter("dbg_q", [128, 2, 4, 512], f32r, isOutput=True)
        dbg_k = nc.declare_dram_parameter("dbg_k", [128, 2, 4, 512], f32r, isOutput=True)
        dbg_v = nc.declare_dram_parameter("dbg_v", [128, 16, 4, C + 1], f32r, isOutput=True)
        dbg_x = nc.declare_dram_parameter("dbg_x", [128, 2, 4, 512], f32r, isOutput=True)
        dbg_rk = nc.declare_dram_parameter("dbg_rk", [128, 16], f32, isOutput=True)
        dbg_rq = nc.declare_dram_parameter("dbg_rq", [1, 2048], f32, isOutput=True)
        dbg_s = nc.declare_dram_parameter("dbg_s", [128, 2, 512], f32, isOutput=True)
        dbg_p = nc.declare_dram_parameter("dbg_p", [128, 2, 512], f32r, isOutput=True)
        dbg_o = nc.declare_dram_parameter("dbg_o", [C + 1, 4, 512], f32, isOutput=True)
        dbg_rd = nc.declare_dram_parameter("dbg_rd", [1, 512], f32, isOutput=True)
        dbg_bc = nc.declare_dram_parameter("dbg_bc", [C, 512], f32, isOutput=True)

    with (
        nc.allow_low_precision(reason="f32r matmul operands by design; fp32 PSUM"),
        tile.TileContext(nc) as tc,
    ):
        with (
            tc.tile_pool(name="singles", bufs=1) as singles,
            tc.tile_pool(name="wts", bufs=2) as wts,
            tc.tile_pool(name="blk", bufs=1 if DEBUG else 2) as blkpool,
            tc.tile_pool(name="sq", bufs=2) as sqpool,
            tc.tile_pool(name="psb", bufs=3) as ppool,
            tc.tile_pool(name="obuf", bufs=2) as obuf,
            tc.tile_pool(name="small", bufs=2) as small,
            tc.tile_pool(name="dram", bufs=1, space="DRAM") as dram,
        ):
            # ---- constants / small inputs ----
            ones_f = singles.tile([128, 64], f32)
            nc.vector.memset(ones_f, 1.0)
            ones1x64 = singles.tile([1, 64], f32)
            nc.vector.memset(ones1x64, 1.0)
            ones1x128 = singles.tile([1, 128], f32)
            nc.vector.memset(ones1x128, 1.0)
            eps_t = singles.tile([128, 1], f32)
            nc.vector.memset(eps_t, EPS)
            invd_t = singles.tile([128, 1], f32)
            nc.vector.memset(invd_t, 1.0 / D)
            ig2q_sb = singles.tile([128, 2], f32r)
            nc.sync.dma_start(out=ig2q_sb, in_=ig2q_d.rearrange("t p -> p t"))
            ig2k_sb = singles.tile([128, 2], f32r)
            nc.sync.dma_start(out=ig2k_sb, in_=ig2k_d.rearrange("t p -> p t"))
            mb_sb = singles.tile([128, 16], f32)
            nc.sync.dma_start(out=mb_sb, in_=mb_d.rearrange("t p -> p t"))

            # weights rotate through 2 pool slots: wq,wk up front; wv,wo reuse
            wq_sb = wts.tile([128, 8, DS], f32r, tag="w")
            wk_sb = wts.tile([128, 8, DS], f32r, tag="w")
            for et in range(8):
                nc.sync.dma_start(out=wq_sb[:, et, :], in_=wqT_d[et * 128 : et * 128 + 128, :])
                nc.scalar.dma_start(out=wk_sb[:, et, :], in_=wkT_d[et * 128 : et * 128 + 128, :])

            # ---- persistent activations ----
            qT = singles.tile([128, 2, 4, 512], f32r)  # [p, dt, nb, n]
            kT = singles.tile([128, 2, 4, 512], f32r)  # [p, dt, mb, m]
            v_sb = singles.tile([128, 16, 4, C + 1], f32r)  # [m_p, mt, h, c|ones]
            xT = singles.tile([128, 2, 4, 512], f32r)  # [p, dt, nb, n]
            nc.vector.tensor_copy(
                v_sb[:, :, :, C], ones_f.rearrange("p (a b) -> p a b", a=16)
            )

            ccq_in = dram.tile([2048], f32)
            ccq_out = dram.tile([2048], f32)
            cck_in = dram.tile([2048], f32)
            cck_out = dram.tile([2048], f32)

            def projection(src_d, w_sb, dst, ig2_sb, cc_in_t, dma_eng):
                """dst[dt, nb] = W'^T-slice @ src-block; partial sumsq -> cc_in."""
                for nb in range(4):
                    blk = blkpool.tile([128, 8, 512], f32r, tag="blk")
                    for et in range(8):
                        dma_eng.dma_start(
                            out=blk[:, et, :],
                            in_=src_d[et * 128 : et * 128 + 128, nb * 512 : nb * 512 + 512],
                        )
                    ss_ps = ssps.tile([1, 512], f32, tag="ss")
                    for dt in range(2):
                        ps = projps.tile([128, 512], f32, tag="proj")
                        for et in range(8):
                            nc.tensor.matmul(
                                ps,
                                w_sb[:, et, dt * 128 : dt * 128 + 128],
                                blk[:, et, :],
                                start=(et == 0),
                                stop=(et == 7),
                            )
                        nc.vector.tensor_copy(dst[:, dt, nb, :], ps)
                        sq = sqpool.tile([128, 512], f32r, tag="sq")
                        nc.vector.tensor_mul(sq, dst[:, dt, nb, :], dst[:, dt, nb, :])
                        nc.tensor.matmul(
                            ss_ps,
                            ig2_sb[:, dt : dt + 1],
                            sq,
                            start=(dt == 0),
                            stop=(dt == 1),
                            skip_group_check=True,
                        )
                    ss_sb = small.tile([1, 512], f32, tag="ss_sb")
                    nc.scalar.copy(ss_sb, ss_ps)
                    nc.sync.dma_start(
                        out=cc_in_t[nb * 512 : nb * 512 + 512].rearrange(
                            "(a n) -> a n", a=1
                        ),
                        in_=ss_sb,
                    )

            def rstd128(cc_out_t, tag):
                """[128,16] lane-parallel rstd chain: p,t -> 1/sqrt(ss/D+eps)."""
                ss128 = small.tile([128, 16], f32, tag=f"ss128{tag}")
                nc.sync.dma_start(
                    out=ss128, in_=cc_out_t.rearrange("(t p) -> p t", p=128)
                )
                std = small.tile([128, 16], f32, tag=f"std{tag}")
                nc.scalar.activation(std, ss128, AF.Sqrt, bias=eps_t, scale=invd_t)
                r = singles.tile([128, 16], f32)
                nc.vector.reciprocal_approx_fast(out=r, in_=std)
                return r

            with (
                tc.tile_pool(name="projps", bufs=2, space="PSUM") as projps,
                tc.tile_pool(name="vps", bufs=2, space="PSUM") as vps,
                tc.tile_pool(name="ssps", bufs=2, space="PSUM") as ssps,
            ):
                # ---- q projection, then its collective (hidden behind k/v) ----
                projection(qT_d, wq_sb, qT, ig2q_sb, ccq_in, nc.sync)
                nc.gpsimd.collective_compute(
                    "AllReduce",
                    mybir.AluOpType.add,
                    replica_groups=[[0, 1, 2, 3], [4, 5, 6, 7]],
                    ins=[ccq_in.opt()],
                    outs=[ccq_out.opt()],
                )

                # ---- k projection, then its collective (hidden behind v) ----
                projection(kfT_d, wk_sb, kT, ig2k_sb, cck_in, nc.scalar)
                nc.gpsimd.collective_compute(
                    "AllReduce",
                    mybir.AluOpType.add,
                    replica_groups=[[0, 1, 2, 3], [4, 5, 6, 7]],
                    ins=[cck_in.opt()],
                    outs=[cck_out.opt()],
                )

                # ---- v projection (kfT re-streamed) ----
                wv_sb = wts.tile([128, 8, DS], f32r, tag="w")
                for et in range(8):
                    nc.scalar.dma_start(out=wv_sb[:, et, :], in_=wvT_d[et * 128 : et * 128 + 128, :])
                for mb in range(4):
                    blk = blkpool.tile([128, 8, 512], f32r, tag="blk")
                    for et in range(8):
                        nc.scalar.dma_start(
                            out=blk[:, et, :],
                            in_=kfT_d[et * 128 : et * 128 + 128, mb * 512 : mb * 512 + 512],
                        )
                    for mt in range(4):
                        psv = vps.tile([128, 256], f32, tag="v")
                        for et in range(8):
                            nc.tensor.matmul(
                                psv,
                                blk[:, et, mt * 128 : mt * 128 + 128],
                                wv_sb[:, et, :],
                                start=(et == 0),
                                stop=(et == 7),
                            )
                        nc.vector.tensor_copy(
                            v_sb[:, mb * 4 + mt, :, 0:C],
                            psv.rearrange("p (h c) -> p h c", c=C),
                        )

                # ---- rstd_k: [128,16] lane-parallel; feeds exp scale directly ----
                rstdk = rstd128(cck_out, "k")

                # ---- rstd_q: row layout [1, 2048] for the bcast outer-products ----
                ssq_row = singles.tile([1, 2048], f32)
                nc.sync.dma_start(
                    out=ssq_row, in_=ccq_out.rearrange("(a n) -> a n", a=1)
                )
                nc.scalar.activation(
                    ssq_row, ssq_row, AF.Sqrt, bias=eps_t[0:1, :], scale=invd_t[0:1, :]
                )
                rs_row = singles.tile([1, 2048], f32)
                nc.vector.reciprocal_approx_fast(out=rs_row, in_=ssq_row)
                # q finalize: qT[d, n] *= rstd_q[n] via ones outer-product bcast
                for nb in range(4):
                    bcq = projps.tile([128, 512], f32, tag="proj")
                    nc.tensor.matmul(
                        bcq,
                        ones1x128,
                        rs_row[:, nb * 512 : nb * 512 + 512],
                        start=True,
                        stop=True,
                    )
                    for dt in range(2):
                        nc.vector.tensor_mul(qT[:, dt, nb, :], qT[:, dt, nb, :], bcq)

                # ---- HAM warm-up burst: ~5us of dense dependency-free matmuls
                # (the collective/norm stall re-throttles the PE clock to 4/8;
                # a fully-busy 3.4us window is needed to flip it back to 8/8
                # before the attention stream, whose fine-grained gaps can
                # never re-warm it)
                warm = projps.tile([128, 512], f32, tag="proj")
                for i in range(20):
                    nc.tensor.matmul(
                        warm,
                        kT[:, 0, 0, 0:128],
                        kT[:, 0, 1, :],
                        start=(i == 0),
                        stop=(i == 19),
                        skip_group_check=True,
                    )
                warm_sink = small.tile([1, 512], f32, tag="rd")
                nc.vector.tensor_copy(warm_sink, warm[0:1, :])

            # ---- phase 3: attention (4 heads) ----
            with (
                tc.tile_pool(name="sps", bufs=2, space="PSUM") as spool,
                tc.tile_pool(name="ops", bufs=1, space="PSUM") as opool,
            ):
                for h in range(4):
                    dt, off = h // 2, (h % 2) * C
                    o4 = opool.tile([C + 1, 4, 512], f32, tag="o4")
                    for mt in range(16):
                        kT_lhs = kT[
                            off : off + C, dt, mt // 4, (mt % 4) * 128 : (mt % 4) * 128 + 128
                        ]
                        pv_halves = []
                        for half in range(2):
                            s2 = spool.tile([128, 2, 512], f32, tag="s2")
                            for i in range(2):
                                nbi = half * 2 + i
                                nc.tensor.matmul(
                                    s2[:, i, :],
                                    kT_lhs,
                                    qT[off : off + C, dt, nbi, :],
                                    start=True,
                                    stop=True,
                                )
                            p2 = ppool.tile([128, 2, 512], f32r, tag="p")
                            nc.scalar.activation(
                                p2, s2, AF.Exp,
                                bias=mb_sb[:, mt : mt + 1],
                                scale=rstdk[:, mt : mt + 1],
                            )
                            pv_halves.append(p2)
                            if DEBUG and h == 0 and mt == 0 and half == 0:
                                s_sb = ppool.tile([128, 2, 512], f32, tag="dbgs")
                                nc.vector.tensor_copy(s_sb, s2)
                                nc.sync.dma_start(out=dbg_s[:], in_=s_sb)
                                nc.sync.dma_start(out=dbg_p[:], in_=p2)
                        for nbi in range(4):
                            nc.tensor.matmul(
                                o4[:, nbi, :],
                                v_sb[:, mt, h, :],
                                pv_halves[nbi // 2][:, nbi % 2, :],
                                start=(mt == 0),
                                stop=(mt == 15),
                                skip_group_check=True,
                            )
                    # free o4 (and its 4 banks) with a single copy so the next
                    # head's PV matmuls aren't blocked behind the normalize
                    o_sb = obuf.tile([C + 1, 4, 512], f32, tag="osb")
                    nc.vector.tensor_copy(o_sb, o4)
                    if DEBUG and h == 0:
                        nc.sync.dma_start(out=dbg_o[:], in_=o_sb)
                    for nb in range(4):
                        den_sb = small.tile([1, 512], f32, tag="den")
                        nc.vector.tensor_copy(den_sb, o_sb[C : C + 1, nb, :])
                        rd = small.tile([1, 512], f32, tag="rd")
                        nc.vector.reciprocal_approx_fast(out=rd, in_=den_sb)
                        bc = spool.tile([128, 2, 512], f32, tag="s2")
                        nc.tensor.matmul(bc[0:C, 0, :], ones1x64, rd, start=True, stop=True)
                        if DEBUG and h == 0 and nb == 0:
                            nc.sync.dma_start(out=dbg_rd[:], in_=rd)
                        nc.vector.tensor_mul(
                            xT[off : off + C, dt, nb, :], o_sb[0:C, nb, :], bc[0:C, 0, :]
                        )

            if DEBUG:
                nc.sync.dma_start(out=dbg_q[:], in_=qT)
                nc.sync.dma_start(out=dbg_k[:], in_=kT)
                nc.sync.dma_start(out=dbg_v[:], in_=v_sb)
                nc.sync.dma_start(out=dbg_x[:], in_=xT)
                nc.sync.dma_start(out=dbg_rk[:], in_=rstdk)
                nc.sync.dma_start(out=dbg_rq[:], in_=rs_row)

            # ---- phase 4: out projection (partial over d-slice) ----
            with tc.tile_pool(name="outps", bufs=3, space="PSUM") as outps:
                wo_sb = wts.tile([128, 2, D], f32r, tag="w")
                for dc in range(2):
                    nc.sync.dma_start(
                        out=wo_sb[:, dc, :], in_=woT_d[dc * 128 : dc * 128 + 128, :]
                    )
                for nb in range(4):
                    for ot in range(8):
                        ps = outps.tile([128, 512], f32, tag="out")
                        for dc in range(2):
                            nc.tensor.matmul(
                                ps,
                                wo_sb[:, dc, ot * 128 : ot * 128 + 128],
                                xT[:, dc, nb, :],
                                start=(dc == 0),
                                stop=(dc == 1),
                            )
                        out_sb = ppool.tile([128, 512], f32, tag="osb")
                        nc.scalar.copy(out_sb, ps)
                        nc.sync.dma_start(
                            out=outT_d[ot * 128 : ot * 128 + 128, nb * 512 : nb * 512 + 512],
                            in_=out_sb,
                        )

    nc.finalize()
    return nc


_NC_CACHE = None


def _get_nc():
    global _NC_CACHE
    if _NC_CACHE is None:
        _NC_CACHE = build()
    return _NC_CACHE


def make_in_maps(querys, key_feats, mask, Wq, Wk, Wv, gq, gk, Wo, bo):
    querys = np.asarray(querys, dtype=np.float32)
    key_feats = np.asarray(key_feats, dtype=np.float32)
    mask = np.asarray(mask)
    gq = np.asarray(gq, dtype=np.float32)
    gk = np.asarray(gk, dtype=np.float32)

    gsq_full = gq * np.float32(SCALE)  # folded into Wq rows
    gsk_full = gk.astype(np.float32)  # folded into Wk rows
    Wq_f = np.asarray(Wq, dtype=np.float32) * gsq_full[:, None]
    Wk_f = np.asarray(Wk, dtype=np.float32) * gsk_full[:, None]

    qT = [round_f32r(querys[b].T) for b in range(B)]
    kfT = [round_f32r(key_feats[b].T) for b in range(B)]
    mb = [
        np.where(mask[b] == 0, np.float32(NEG), np.float32(0.0))
        .astype(np.float32)
        .reshape(16, 128)
        for b in range(B)
    ]
    wqT, wkT, wvT, woT, ig2q, ig2k = [], [], [], [], [], []
    for j in range(4):
        dsl = slice(j * DS, (j + 1) * DS)
        wqT.append(round_f32r(Wq_f[dsl].T))
        wkT.append(round_f32r(Wk_f[dsl].T))
        wvT.append(round_f32r(np.asarray(Wv)[dsl].T))
        woT.append(round_f32r(np.asarray(Wo)[:, dsl].T))
        # sumsq compensation: raw sumsq = sum_d (q'_d)^2 / gs_d^2
        ig2q.append(round_f32r((1.0 / gsq_full[dsl] ** 2).reshape(2, 128)))
        ig2k.append(round_f32r((1.0 / gsk_full[dsl] ** 2).reshape(2, 128)))

    in_maps = []
    for cid in range(NCORES):
        b, j = cid // 4, cid % 4
        in_maps.append(
            {
                "qT": qT[b],
                "kfT": kfT[b],
                "wqT": wqT[j],
                "wkT": wkT[j],
                "wvT": wvT[j],
                "woT": woT[j],
                "ig2q": ig2q[j],
                "ig2k": ig2k[j],
                "mbias": mb[b],
            }
        )
    return in_maps


def assemble(results, bo):
    bo = np.asarray(bo, dtype=np.float32)
    out = np.zeros((B, N, D), dtype=np.float32)
    for cid in range(NCORES):
        b = cid // 4
        out[b] += results[cid]["outT"].T
    out += bo
    return out


def kernel(querys, key_feats, mask, Wq, Wk, Wv, gq, gk, Wo, bo):
    nc = _get_nc()
    in_maps = make_in_maps(querys, key_feats, mask, Wq, Wk, Wv, gq, gk, Wo, bo)
    res = run_bass_kernel_spmd(nc, in_maps, list(range(NCORES)))
    return assemble(res.results, bo)


# revision 35
# speedup vs baseline: 1.1731x; 1.0185x over previous
"""CrossAttention Trainium2 kernel (8 NeuronCores).

Reference computation (B=2, N=M=2048, D=1024, H=16, C=64):
    q = rmsnorm(querys @ Wq.T, gq) * C**-0.5       [B,N,D]
    k = rmsnorm(key_feats @ Wk.T, gk)              [B,M,D]
    v = key_feats @ Wv.T                           [B,M,D]
    attn = softmax(mask(q @ k.T per head))         [B,H,N,M]
    out = (attn @ v per head, concat) @ Wo.T + bo  [B,N,D]

Sharding: core = b*4 + j (b in {0,1}; j in {0..3} owns heads 4j..4j+3 = a
256-wide slice of D). Host pre-transposes inputs/weights, folds gq*scale /
gk into Wq / Wk rows, and pre-rounds everything to f32r (fp32 with 11-bit
mantissa -> full PE rate). Per core:

  - q'^T / k'^T projections in d-slice layout [256, 2048] (contraction over
    E in the partition dim), v in [2048, 256]. q' = gs_q * q_raw etc.
  - rmsnorm sum-of-squares over the FULL D: per-core partial sumsq is
    computed by a matmul against a 1/gs^2-weighted column (compensating the
    folded gains) and AllReduced (8KB) across the 4 cores of each b; the
    collectives are emitted right after their producing phase so they hide
    behind the next projection.
  - rstd chains run lane-parallel in [128,16] layout. rstd_k is NOT applied
    to k': in the S^T = k'q'^T orientation the softmax logit scale rstd_k[m]
    is per-partition, so it folds into the exp ACTIVATE as its scale operand
    (and the mask as its bias: 0 / -1e30). rstd_q is applied to q' via a
    PE-transpose into row layout + ones outer-product broadcast.
  - attention per head: for each m-tile, 4 QK matmuls (one per 512-wide
    n-block, shared k stationary) -> batched exp -> 4 PV matmuls into a
    4-bank accumulator. v carries a 65th column of ones so row 64 of the
    accumulator is the softmax denominator (reciprocal_approx_fast + ones
    outer-product broadcast + one multiply normalizes the head output).
  - out projection produces a partial out^T [1024, 2048] (contraction over
    this core's d-slice only); the host sums 4 partials per b and adds bo.
"""

import os

import numpy as np

import concourse.tile as tile
from concourse import bacc, mybir
from concourse.bass_utils import run_bass_kernel_spmd

DEBUG = bool(os.environ.get("BASSK_DEBUG"))

B, N, M, D, H = 2, 2048, 2048, 1024, 16
C = D // H  # 64, head dim
E = D  # input feature dim
EPS = 1e-6
SCALE = C ** (-0.5)
DS = D // 4  # 256, per-core d-slice
NCORES = 8

f32 = mybir.dt.float32
f32r = mybir.dt.float32r
AF = mybir.ActivationFunctionType

NEG = -1e30


def round_f32r(x: np.ndarray) -> np.ndarray:
    b = np.ascontiguousarray(x, dtype=np.float32).view(np.uint32)
    b = (b + 0x800) & np.uint32(0xFFFFF000)
    return b.view(np.float32)


def build():
    nc = bacc.Bacc(None, target_bir_lowering=False)

    qT_d = nc.declare_dram_parameter("qT", [E, N], f32r, isOutput=False)
    kfT_d = nc.declare_dram_parameter("kfT", [E, M], f32r, isOutput=False)
    wqT_d = nc.declare_dram_parameter("wqT", [E, DS], f32r, isOutput=False)
    wkT_d = nc.declare_dram_parameter("wkT", [E, DS], f32r, isOutput=False)
    wvT_d = nc.declare_dram_parameter("wvT", [E, DS], f32r, isOutput=False)
    woT_d = nc.declare_dram_parameter("woT", [DS, D], f32r, isOutput=False)
    ig2q_d = nc.declare_dram_parameter("ig2q", [2, 128], f32r, isOutput=False)
    ig2k_d = nc.declare_dram_parameter("ig2k", [2, 128], f32r, isOutput=False)
    mb_d = nc.declare_dram_parameter("mbias", [16, 128], f32, isOutput=False)
    outT_d = nc.declare_dram_parameter("outT", [D, N], f32, isOutput=True)
    if DEBUG:
        dbg_q = nc.declare_dram_parameter("dbg_q", [128, 2, 4, 512], f32r, isOutput=True)
        dbg_k = nc.declare_dram_parameter("dbg_k", [128, 2, 4, 512], f32r, isOutput=True)
        dbg_v = nc.declare_dram_parameter("dbg_v", [128, 16, 4, C + 1], f32r, isOutput=True)
        dbg_x = nc.declare_dram_parameter("dbg_x", [128, 2, 4, 512], f32r, isOutput=True)
        dbg_rk = nc.declare_dram_parameter("dbg_rk", [128, 16], f32, isOutput=True)
        dbg_rq = nc.declare_dram_parameter("dbg_rq", [1, 2048], f32, isOutput=True)
        dbg_s = nc.declare_dram_parameter("dbg_s", [128, 2, 512], f32, isOutput=True)
        dbg_p = nc.declare_dram_parameter("dbg_p", [128, 2, 512], f32r, isOutput=True)
        dbg_o = nc.declare_dram_parameter("dbg_o", [C + 1, 4, 512], f32, isOutput=True)
        dbg_rd = nc.declare_dram_parameter("dbg_rd", [1, 512], f32, isOutput=True)
        dbg_bc = nc.declare_dram_parameter("dbg_bc", [C, 512], f32, isOutput=True)

    with (
        nc.allow_low_precision(reason="f32r matmul operands by design; fp32 PSUM"),
        tile.TileContext(nc) as tc,
    ):
        with (
            tc.tile_pool(name="singles", bufs=1) as singles,
            tc.tile_pool(name="wts", bufs=2) as wts,
            tc.tile_pool(name="blk", bufs=1 if DEBUG else 2) as blkpool,
            tc.tile_pool(name="sq", bufs=2) as sqpool,
            tc.tile_pool(name="psb", bufs=3) as ppool,
            tc.tile_pool(name="obuf", bufs=2) as obuf,
            tc.tile_pool(name="rdp", bufs=8) as rdp,
            tc.tile_pool(name="small", bufs=2) as small,
            tc.tile_pool(name="dram", bufs=1, space="DRAM") as dram,
        ):
            # ---- constants / small inputs ----
            ones_f = singles.tile([128, 64], f32)
            nc.vector.memset(ones_f, 1.0)
            ones1x64 = singles.tile([1, 64], f32)
            nc.vector.memset(ones1x64, 1.0)
            ones1x128 = singles.tile([1, 128], f32)
            nc.vector.memset(ones1x128, 1.0)
            eps_t = singles.tile([128, 1], f32)
            nc.vector.memset(eps_t, EPS)
            invd_t = singles.tile([128, 1], f32)
            nc.vector.memset(invd_t, 1.0 / D)
            ig2q_sb = singles.tile([128, 2], f32r)
            nc.sync.dma_start(out=ig2q_sb, in_=ig2q_d.rearrange("t p -> p t"))
            ig2k_sb = singles.tile([128, 2], f32r)
            nc.sync.dma_start(out=ig2k_sb, in_=ig2k_d.rearrange("t p -> p t"))
            mb_sb = singles.tile([128, 16], f32)
            nc.sync.dma_start(out=mb_sb, in_=mb_d.rearrange("t p -> p t"))

            # weights rotate through 2 pool slots: wq,wk up front; wv,wo reuse
            wq_sb = wts.tile([128, 8, DS], f32r, tag="w")
            wk_sb = wts.tile([128, 8, DS], f32r, tag="w")
            for et in range(8):
                nc.sync.dma_start(out=wq_sb[:, et, :], in_=wqT_d[et * 128 : et * 128 + 128, :])
                nc.scalar.dma_start(out=wk_sb[:, et, :], in_=wkT_d[et * 128 : et * 128 + 128, :])

            # ---- persistent activations ----
            qT = singles.tile([128, 2, 4, 512], f32r)  # [p, dt, nb, n]
            kT = singles.tile([128, 2, 4, 512], f32r)  # [p, dt, mb, m]
            v_sb = singles.tile([128, 16, 4, C + 1], f32r)  # [m_p, mt, h, c|ones]
            xT = singles.tile([128, 2, 4, 512], f32r)  # [p, dt, nb, n]
            nc.vector.tensor_copy(
                v_sb[:, :, :, C], ones_f.rearrange("p (a b) -> p a b", a=16)
            )

            ccq_in = dram.tile([2048], f32)
            ccq_out = dram.tile([2048], f32)
            cck_in = dram.tile([2048], f32)
            cck_out = dram.tile([2048], f32)

            def projection(src_d, w_sb, dst, ig2_sb, cc_in_t, dma_eng):
                """dst[dt, nb] = W'^T-slice @ src-block; partial sumsq -> cc_in."""
                for nb in range(4):
                    blk = blkpool.tile([128, 8, 512], f32r, tag="blk")
                    for et in range(8):
                        dma_eng.dma_start(
                            out=blk[:, et, :],
                            in_=src_d[et * 128 : et * 128 + 128, nb * 512 : nb * 512 + 512],
                        )
                    ss_ps = ssps.tile([1, 512], f32, tag="ss")
                    for dt in range(2):
                        ps = projps.tile([128, 512], f32, tag="proj")
                        for et in range(8):
                            nc.tensor.matmul(
                                ps,
                                w_sb[:, et, dt * 128 : dt * 128 + 128],
                                blk[:, et, :],
                                start=(et == 0),
                                stop=(et == 7),
                            )
                        nc.vector.tensor_copy(dst[:, dt, nb, :], ps)
                        sq = sqpool.tile([128, 512], f32r, tag="sq")
                        nc.vector.tensor_mul(sq, dst[:, dt, nb, :], dst[:, dt, nb, :])
                        nc.tensor.matmul(
                            ss_ps,
                            ig2_sb[:, dt : dt + 1],
                            sq,
                            start=(dt == 0),
                            stop=(dt == 1),
                            skip_group_check=True,
                        )
                    ss_sb = small.tile([1, 512], f32, tag="ss_sb")
                    nc.scalar.copy(ss_sb, ss_ps)
                    nc.sync.dma_start(
                        out=cc_in_t[nb * 512 : nb * 512 + 512].rearrange(
                            "(a n) -> a n", a=1
                        ),
                        in_=ss_sb,
                    )

            def rstd128(cc_out_t, tag):
                """[128,16] lane-parallel rstd chain: p,t -> 1/sqrt(ss/D+eps)."""
                ss128 = small.tile([128, 16], f32, tag=f"ss128{tag}")
                nc.sync.dma_start(
                    out=ss128, in_=cc_out_t.rearrange("(t p) -> p t", p=128)
                )
                std = small.tile([128, 16], f32, tag=f"std{tag}")
                nc.scalar.activation(std, ss128, AF.Sqrt, bias=eps_t, scale=invd_t)
                r = singles.tile([128, 16], f32)
                nc.vector.reciprocal_approx_fast(out=r, in_=std)
                return r

            with (
                tc.tile_pool(name="projps", bufs=2, space="PSUM") as projps,
                tc.tile_pool(name="vps", bufs=2, space="PSUM") as vps,
                tc.tile_pool(name="ssps", bufs=2, space="PSUM") as ssps,
            ):
                # ---- q projection, then its collective (hidden behind k/v) ----
                projection(qT_d, wq_sb, qT, ig2q_sb, ccq_in, nc.sync)
                nc.gpsimd.collective_compute(
                    "AllReduce",
                    mybir.AluOpType.add,
                    replica_groups=[[0, 1, 2, 3], [4, 5, 6, 7]],
                    ins=[ccq_in.opt()],
                    outs=[ccq_out.opt()],
                )

                # ---- k projection, then its collective (hidden behind v) ----
                projection(kfT_d, wk_sb, kT, ig2k_sb, cck_in, nc.scalar)
                nc.gpsimd.collective_compute(
                    "AllReduce",
                    mybir.AluOpType.add,
                    replica_groups=[[0, 1, 2, 3], [4, 5, 6, 7]],
                    ins=[cck_in.opt()],
                    outs=[cck_out.opt()],
                )

                # ---- v projection (kfT re-streamed) ----
                wv_sb = wts.tile([128, 8, DS], f32r, tag="w")
                for et in range(8):
                    nc.scalar.dma_start(out=wv_sb[:, et, :], in_=wvT_d[et * 128 : et * 128 + 128, :])
                for mb in range(4):
                    blk = blkpool.tile([128, 8, 512], f32r, tag="blk")
                    for et in range(8):
                        nc.scalar.dma_start(
                            out=blk[:, et, :],
                            in_=kfT_d[et * 128 : et * 128 + 128, mb * 512 : mb * 512 + 512],
                        )
                    for mt in range(4):
                        psv = vps.tile([128, 256], f32, tag="v")
                        for et in range(8):
                            nc.tensor.matmul(
                                psv,
                                blk[:, et, mt * 128 : mt * 128 + 128],
                                wv_sb[:, et, :],
                                start=(et == 0),
                                stop=(et == 7),
                            )
                        nc.vector.tensor_copy(
                            v_sb[:, mb * 4 + mt, :, 0:C],
                            psv.rearrange("p (h c) -> p h c", c=C),
                        )

                # ---- rstd_k: [128,16] lane-parallel; feeds exp scale directly ----
                rstdk = rstd128(cck_out, "k")

                # ---- rstd_q: row layout [1, 2048] for the bcast outer-products ----
                ssq_row = singles.tile([1, 2048], f32)
                nc.sync.dma_start(
                    out=ssq_row, in_=ccq_out.rearrange("(a n) -> a n", a=1)
                )
                nc.scalar.activation(
                    ssq_row, ssq_row, AF.Sqrt, bias=eps_t[0:1, :], scale=invd_t[0:1, :]
                )
                rs_row = singles.tile([1, 2048], f32)
                nc.vector.reciprocal_approx_fast(out=rs_row, in_=ssq_row)
                # q finalize: qT[d, n] *= rstd_q[n] via ones outer-product bcast
                for nb in range(4):
                    bcq = projps.tile([128, 512], f32, tag="proj")
                    nc.tensor.matmul(
                        bcq,
                        ones1x128,
                        rs_row[:, nb * 512 : nb * 512 + 512],
                        start=True,
                        stop=True,
                    )
                    for dt in range(2):
                        nc.vector.tensor_mul(qT[:, dt, nb, :], qT[:, dt, nb, :], bcq)

                # ---- HAM warm-up burst: ~5us of dense dependency-free matmuls
                # (the collective/norm stall re-throttles the PE clock to 4/8;
                # a fully-busy 3.4us window is needed to flip it back to 8/8
                # before the attention stream, whose fine-grained gaps can
                # never re-warm it)
                warm = projps.tile([128, 512], f32, tag="proj")
                for i in range(20):
                    nc.tensor.matmul(
                        warm,
                        kT[:, 0, 0, 0:128],
                        kT[:, 0, 1, :],
                        start=(i == 0),
                        stop=(i == 19),
                        skip_group_check=True,
                    )
                warm_sink = small.tile([1, 512], f32, tag="rd")
                nc.vector.tensor_copy(warm_sink, warm[0:1, :])

            # ---- phase 3: attention (4 heads) ----
            with (
                tc.tile_pool(name="sps", bufs=2, space="PSUM") as spool,
                tc.tile_pool(name="ops", bufs=1, space="PSUM") as opool,
            ):
                def emit_normalize(state):
                    """bc outer-products + muls for a head whose DVE recips are
                    done by now (emitted one head late to keep PE gapless)."""
                    hh, oo_sb, rds = state
                    ddt, ooff = hh // 2, (hh % 2) * C
                    for nb in range(4):
                        bc = spool.tile([128, 2, 512], f32, tag="s2")
                        nc.tensor.matmul(
                            bc[0:C, 0, :], ones1x64, rds[nb], start=True, stop=True
                        )
                        nc.vector.tensor_mul(
                            xT[ooff : ooff + C, ddt, nb, :],
                            oo_sb[0:C, nb, :],
                            bc[0:C, 0, :],
                        )

                prev = None
                for h in range(4):
                    dt, off = h // 2, (h % 2) * C
                    o4 = opool.tile([C + 1, 4, 512], f32, tag="o4")
                    for mt in range(16):
                        kT_lhs = kT[
                            off : off + C, dt, mt // 4, (mt % 4) * 128 : (mt % 4) * 128 + 128
                        ]
                        pv_halves = []
                        for half in range(2):
                            s2 = spool.tile([128, 2, 512], f32, tag="s2")
                            for i in range(2):
                                nbi = half * 2 + i
                                nc.tensor.matmul(
                                    s2[:, i, :],
                                    kT_lhs,
                                    qT[off : off + C, dt, nbi, :],
                                    start=True,
                                    stop=True,
                                )
                            p2 = ppool.tile([128, 2, 512], f32r, tag="p")
                            nc.scalar.activation(
                                p2, s2, AF.Exp,
                                bias=mb_sb[:, mt : mt + 1],
                                scale=rstdk[:, mt : mt + 1],
                            )
                            pv_halves.append(p2)
                            if DEBUG and h == 0 and mt == 0 and half == 0:
                                s_sb = ppool.tile([128, 2, 512], f32, tag="dbgs")
                                nc.vector.tensor_copy(s_sb, s2)
                                nc.sync.dma_start(out=dbg_s[:], in_=s_sb)
                                nc.sync.dma_start(out=dbg_p[:], in_=p2)
                        for nbi in range(4):
                            nc.tensor.matmul(
                                o4[:, nbi, :],
                                v_sb[:, mt, h, :],
                                pv_halves[nbi // 2][:, nbi % 2, :],
                                start=(mt == 0),
                                stop=(mt == 15),
                                skip_group_check=True,
                            )
                    # free o4 (and its 4 banks) with a single copy so the next
                    # head's PV matmuls aren't blocked behind the normalize
                    o_sb = obuf.tile([C + 1, 4, 512], f32, tag="osb")
                    nc.vector.tensor_copy(o_sb, o4)
                    if DEBUG and h == 0:
                        nc.sync.dma_start(out=dbg_o[:], in_=o_sb)
                    rds = []
                    for nb in range(4):
                        den_sb = rdp.tile([1, 512], f32, tag="den")
                        nc.vector.tensor_copy(den_sb, o_sb[C : C + 1, nb, :])
                        rd = rdp.tile([1, 512], f32, tag="rd")
                        nc.vector.reciprocal_approx_fast(out=rd, in_=den_sb)
                        rds.append(rd)
                    if prev is not None:
                        emit_normalize(prev)
                    prev = (h, o_sb, rds)
                emit_normalize(prev)

            if DEBUG:
                nc.sync.dma_start(out=dbg_q[:], in_=qT)
                nc.sync.dma_start(out=dbg_k[:], in_=kT)
                nc.sync.dma_start(out=dbg_v[:], in_=v_sb)
                nc.sync.dma_start(out=dbg_x[:], in_=xT)
                nc.sync.dma_start(out=dbg_rk[:], in_=rstdk)
                nc.sync.dma_start(out=dbg_rq[:], in_=rs_row)

            # ---- phase 4: out projection (partial over d-slice) ----
            with tc.tile_pool(name="outps", bufs=3, space="PSUM") as outps:
                wo_sb = wts.tile([128, 2, D], f32r, tag="w")
                for dc in range(2):
                    nc.sync.dma_start(
                        out=wo_sb[:, dc, :], in_=woT_d[dc * 128 : dc * 128 + 128, :]
                    )
                for nb in range(4):
                    for ot in range(8):
                        ps = outps.tile([128, 512], f32, tag="out")
                        for dc in range(2):
                            nc.tensor.matmul(
                                ps,
                                wo_sb[:, dc, ot * 128 : ot * 128 + 128],
                                xT[:, dc, nb, :],
                                start=(dc == 0),
                                stop=(dc == 1),
                            )
                        out_sb = ppool.tile([128, 512], f32, tag="osb")
                        nc.scalar.copy(out_sb, ps)
                        nc.sync.dma_start(
                            out=outT_d[ot * 128 : ot * 128 + 128, nb * 512 : nb * 512 + 512],
                            in_=out_sb,
                        )

    nc.finalize()
    return nc


_NC_CACHE = None


def _get_nc():
    global _NC_CACHE
    if _NC_CACHE is None:
        _NC_CACHE = build()
    return _NC_CACHE


def make_in_maps(querys, key_feats, mask, Wq, Wk, Wv, gq, gk, Wo, bo):
    querys = np.asarray(querys, dtype=np.float32)
    key_feats = np.asarray(key_feats, dtype=np.float32)
    mask = np.asarray(mask)
    gq = np.asarray(gq, dtype=np.float32)
    gk = np.asarray(gk, dtype=np.float32)

    gsq_full = gq * np.float32(SCALE)  # folded into Wq rows
    gsk_full = gk.astype(np.float32)  # folded into Wk rows
    Wq_f = np.asarray(Wq, dtype=np.float32) * gsq_full[:, None]
    Wk_f = np.asarray(Wk, dtype=np.float32) * gsk_full[:, None]

    qT = [round_f32r(querys[b].T) for b in range(B)]
    kfT = [round_f32r(key_feats[b].T) for b in range(B)]
    mb = [
        np.where(mask[b] == 0, np.float32(NEG), np.float32(0.0))
        .astype(np.float32)
        .reshape(16, 128)
        for b in range(B)
    ]
    wqT, wkT, wvT, woT, ig2q, ig2k = [], [], [], [], [], []
    for j in range(4):
        dsl = slice(j * DS, (j + 1) * DS)
        wqT.append(round_f32r(Wq_f[dsl].T))
        wkT.append(round_f32r(Wk_f[dsl].T))
        wvT.append(round_f32r(np.asarray(Wv)[dsl].T))
        woT.append(round_f32r(np.asarray(Wo)[:, dsl].T))
        # sumsq compensation: raw sumsq = sum_d (q'_d)^2 / gs_d^2
        ig2q.append(round_f32r((1.0 / gsq_full[dsl] ** 2).reshape(2, 128)))
        ig2k.append(round_f32r((1.0 / gsk_full[dsl] ** 2).reshape(2, 128)))

    in_maps = []
    for cid in range(NCORES):
        b, j = cid // 4, cid % 4
        in_maps.append(
            {
                "qT": qT[b],
                "kfT": kfT[b],
                "wqT": wqT[j],
                "wkT": wkT[j],
                "wvT": wvT[j],
                "woT": woT[j],
                "ig2q": ig2q[j],
                "ig2k": ig2k[j],
                "mbias": mb[b],
            }
        )
    return in_maps


def assemble(results, bo):
    bo = np.asarray(bo, dtype=np.float32)
    out = np.zeros((B, N, D), dtype=np.float32)
    for cid in range(NCORES):
        b = cid // 4
        out[b] += results[cid]["outT"].T
    out += bo
    return out


def kernel(querys, key_feats, mask, Wq, Wk, Wv, gq, gk, Wo, bo):
    nc = _get_nc()
    in_maps = make_in_maps(querys, key_feats, mask, Wq, Wk, Wv, gq, gk, Wo, bo)
    res = run_bass_kernel_spmd(nc, in_maps, list(range(NCORES)))
    return assemble(res.results, bo)
